# revision 18
# baseline (speedup 1.0000x reference)
"""Trainium2 Bass kernel for nn_AlsoDecoder (gnn message passing edge MLP).

reference math:
    pos = also_pts[row] - pcd[col]
    x   = concat([feats[col], pos]) @ w_in + b_in
    x   = relu(x) @ w1 + b1
    x   = relu(x) @ w2 + b2
    x   = x @ w_out + b_out
    probs = softmax(x, axis=1)          # OUT = 2
    return probs, also_labels[row]

Kernel strategy (8 cores, edge-parallel):
  - Algebraic folding:
      L1 per-edge matmul becomes per-node tables:
        S[n] = feats[n]@w_in[:64] - pcd[n]@w_in[64:67]      (source nodes)
        Q[m] = also_pts[m]@w_in[64:67] + b_in               (query nodes)
        x1[e] = S[col[e]] + Q[row[e]]
      L3+L4 are linear back-to-back (no relu between), and softmax over 2
      classes is a sigmoid of the logit difference:
        d = relu(y2) @ c + c0,  c = w2@(w_out[:,1]-w_out[:,0])
        probs = [sigmoid(-d), sigmoid(+d)]
  - Device phase A builds the bf16 S/Q tables (Q carries the label in col 64).
  - Device phase B: per 12288-edge supergroup, one SWDGE indirect gather for
    S[col] and one for Q[row] with CCE-add fused (x1 = S+Q straight from DMA),
    relu on DVE into a compact tile, PE transpose to feature-major with 2
    edge-blocks packed into 128 partitions, one block-diagonal 64x64 matmul,
    ACT relu+bias, a block-diag +/-dot matmul (4 chunks stacked into one PSUM
    bank at positions 0/32/64/96), one ACT sigmoid per 4096 edges emitting
    both probabilities, one DMA of the whole bank to DRAM (host extracts the
    16 used rows), and a label side-channel via a small PE transpose.
"""

import numpy as np
import ml_dtypes

BF16 = ml_dtypes.bfloat16

# problem shape (hardcoded per spec)
N = 100000
M = 100000
E = 2000000
NCORES = 8

# layout constants
NODE_GROUPS = 100
NPAD = NODE_GROUPS * 1024   # 102400
F_SLOT = 8                  # nodes per partition per precompute group
GBATCH = 4                  # precompute groups per DMA batch
ROW_S = 64                  # S table row elems (bf16) = 128B
ROW_Q = 65                  # Q table row elems (bf16): 64 feats + label
SLOTS = 96                  # 128-edge slots per supergroup
SG_EDGES = SLOTS * 128      # 12288
WINDOWS_PER_SG = 3          # 4096-edge sigmoid windows
N_SG = 21
E_PAD = N_SG * SG_EDGES     # 258048
E_SHARD = E // NCORES       # 250000

_CACHE = {}


def _build(n_sg=N_SG, node_groups=NODE_GROUPS, debug_taps=False):
    """Build + compile the per-core Bass graph."""
    import concourse.bass as bass
    import concourse.tile as tile
    from concourse import bacc, mybir
    from concourse.bass import AP, IndirectOffsetOnAxis
    from concourse.masks import make_identity

    dt = mybir.dt
    e_pad = n_sg * SG_EDGES
    npad = node_groups * 1024
    n_win = n_sg * WINDOWS_PER_SG
    assert node_groups % GBATCH == 0

    nc = bacc.Bacc("TRN2", target_bir_lowering=False, debug=False,
                   num_devices=NCORES)

    featsT = nc.dram_tensor("featsT", [67, npad], dt.bfloat16,
                            kind="ExternalInput").ap()
    alsoT = nc.dram_tensor("alsoT", [5, npad], dt.bfloat16,
                           kind="ExternalInput").ap()
    wext = nc.dram_tensor("wext", [67, 64], dt.bfloat16,
                          kind="ExternalInput").ap()
    wq = nc.dram_tensor("wq", [5, ROW_Q], dt.bfloat16,
                        kind="ExternalInput").ap()
    w1blk = nc.dram_tensor("w1blk", [128, 128], dt.bfloat16,
                           kind="ExternalInput").ap()
    b1blk = nc.dram_tensor("b1blk", [128, 1], dt.float32,
                           kind="ExternalInput").ap()
    cblk = nc.dram_tensor("cblk", [128, 32], dt.bfloat16,
                          kind="ExternalInput").ap()
    sigbias = nc.dram_tensor("sigbias", [128, 1], dt.float32,
                             kind="ExternalInput").ap()
    colx = nc.dram_tensor("colx", [n_sg, 128, SLOTS], dt.int32,
                          kind="ExternalInput").ap()
    rowx = nc.dram_tensor("rowx", [n_sg, 128, SLOTS], dt.int32,
                          kind="ExternalInput").ap()
    oscr = nc.dram_tensor("oscr", [n_win, 128, 512], dt.float32,
                          kind="ExternalOutput").ap()
    labo = nc.dram_tensor("labo", [e_pad], dt.float32,
                          kind="ExternalOutput").ap()

    tab_kind = "ExternalOutput" if debug_taps else "Internal"
    s_tab = nc.dram_tensor("s_tab", [npad, ROW_S], dt.bfloat16,
                           kind=tab_kind).ap()
    q_tab = nc.dram_tensor("q_tab", [npad, ROW_Q], dt.bfloat16,
                           kind=tab_kind).ap()
    if debug_taps:
        xgdbg = nc.dram_tensor("xgdbg", [128, SLOTS * ROW_Q], dt.bfloat16,
                               kind="ExternalOutput").ap()

    def dram_view(ap, offset_elems, dims):
        return AP(ap.tensor, offset_elems, list(dims))

    RELU = mybir.ActivationFunctionType.Relu
    SIGM = mybir.ActivationFunctionType.Sigmoid

    with tile.TileContext(nc) as tc:
        with tc.tile_pool(name="const", bufs=1) as cpool:
            ident = cpool.tile([128, 128], dt.bfloat16)
            make_identity(nc, ident[:])
            w1t = cpool.tile([128, 128], dt.bfloat16)
            nc.sync.dma_start(w1t[:], w1blk[:])
            cbt = cpool.tile([128, 32], dt.bfloat16)
            nc.sync.dma_start(cbt[:], cblk[:])
            b1t = cpool.tile([128, 1], dt.float32)
            nc.sync.dma_start(b1t[:], b1blk[:])
            sbt = cpool.tile([128, 1], dt.float32)
            nc.sync.dma_start(sbt[:], sigbias[:])
            wet = cpool.tile([67, 64], dt.bfloat16)
            nc.sync.dma_start(wet[:], wext[:])
            wqt = cpool.tile([5, ROW_Q], dt.bfloat16)
            nc.sync.dma_start(wqt[:], wq[:])

            # ================= phase A: build S/Q tables =================
            with tc.tile_pool(name="pcin", bufs=2) as pcin, \
                 tc.tile_pool(name="pcps", bufs=2, space="PSUM") as pcps, \
                 tc.tile_pool(name="pcst", bufs=2) as pcst:
                for gb in range(node_groups // GBATCH):
                    n0 = gb * GBATCH * 1024
                    fst = pcin.tile([67, 128, GBATCH * F_SLOT], dt.bfloat16,
                                    tag="fst")
                    nc.sync.dma_start(fst[:], featsT[:, n0:n0 + GBATCH * 1024])
                    ast = pcin.tile([5, 128, GBATCH * F_SLOT], dt.bfloat16,
                                    tag="ast")
                    nc.sync.dma_start(ast[:], alsoT[:, n0:n0 + GBATCH * 1024])

                    sst = pcst.tile([128, GBATCH, F_SLOT * 64], dt.bfloat16,
                                    tag="sst")
                    qst = pcst.tile([128, GBATCH, F_SLOT, ROW_Q], dt.bfloat16,
                                    tag="qst")
                    half = F_SLOT // 2
                    for g in range(GBATCH):
                        sps = pcps.tile([128, F_SLOT, 64], dt.float32,
                                        tag="sps")
                        qps1 = pcps.tile([128, half, ROW_Q], dt.float32,
                                         tag="qps1")
                        qps2 = pcps.tile([128, half, ROW_Q], dt.float32,
                                         tag="qps2")
                        for m in range(F_SLOT):
                            gm = g * F_SLOT + m
                            nc.tensor.matmul(sps[:, m, :], lhsT=fst[:, :, gm],
                                             rhs=wet[:], start=True, stop=True)
                            qp = qps1 if m < half else qps2
                            nc.tensor.matmul(qp[:, m % half, :],
                                             lhsT=ast[:, :, gm],
                                             rhs=wqt[:], start=True, stop=True)
                        nc.scalar.copy(sst[:, g, :], sps[:, :, :])
                        nc.scalar.copy(qst[:, g, 0:half, :], qps1[:, :, :])
                        nc.scalar.copy(qst[:, g, half:F_SLOT, :],
                                       qps2[:, :, :])

                    # node (gb, g, p, m) -> row n0 + g*1024 + p*8 + m
                    nc.sync.dma_start(
                        out=dram_view(s_tab, n0 * ROW_S,
                                      [(1024 * ROW_S, GBATCH),
                                       (F_SLOT * ROW_S, 128),
                                       (1, F_SLOT * ROW_S)]),
                        in_=AP(sst[:].tensor, sst[:].offset,
                               [(GBATCH * F_SLOT * 64, 128),
                                (F_SLOT * 64, GBATCH), (1, F_SLOT * 64)]))
                    nc.sync.dma_start(
                        out=dram_view(q_tab, n0 * ROW_Q,
                                      [(1024 * ROW_Q, GBATCH),
                                       (F_SLOT * ROW_Q, 128),
                                       (1, F_SLOT * ROW_Q)]),
                        in_=AP(qst[:].tensor, qst[:].offset,
                               [(GBATCH * F_SLOT * ROW_Q, 128),
                                (F_SLOT * ROW_Q, GBATCH),
                                (1, F_SLOT * ROW_Q)]))

            # ================= phase B: edges =================
            s_view = dram_view(s_tab, 0, [(ROW_S, npad), (1, ROW_S)])
            q_view = dram_view(q_tab, 0, [(ROW_Q, npad), (1, ROW_Q)])

            with tc.tile_pool(name="xg", bufs=2) as xgp, \
                 tc.tile_pool(name="xf", bufs=2) as xfp, \
                 tc.tile_pool(name="idx", bufs=2) as idxp, \
                 tc.tile_pool(name="tps", bufs=2, space="PSUM") as tpsp, \
                 tc.tile_pool(name="x1t", bufs=2) as x1tp, \
                 tc.tile_pool(name="y2", bufs=2, space="PSUM") as y2p, \
                 tc.tile_pool(name="h2", bufs=2) as h2p, \
                 tc.tile_pool(name="sgps", bufs=2, space="PSUM") as sgp, \
                 tc.tile_pool(name="sout", bufs=2) as sop, \
                 tc.tile_pool(name="lab", bufs=2) as lbp, \
                 tc.tile_pool(name="labps", bufs=1, space="PSUM") as ltp:

                sgps = None
                for sg in range(n_sg):
                    e0 = sg * SG_EDGES
                    cix = idxp.tile([128, SLOTS], dt.int32, tag="cix")
                    nc.sync.dma_start(cix[:], colx[sg])
                    rix = idxp.tile([128, SLOTS], dt.int32, tag="rix")
                    nc.sync.dma_start(rix[:], rowx[sg])

                    xg = xgp.tile([128, SLOTS, ROW_Q], dt.bfloat16, tag="xg")
                    nc.vector.memset(xg[:, :, 64:65], 0.0)
                    for sl in range(SLOTS):
                        nc.gpsimd.indirect_dma_start(
                            out=xg[:, sl, 0:64], out_offset=None,
                            in_=s_view,
                            in_offset=IndirectOffsetOnAxis(ap=cix[:, sl:sl + 1],
                                                           axis=0))
                        nc.gpsimd.indirect_dma_start(
                            out=xg[:, sl, :], out_offset=None,
                            in_=q_view,
                            in_offset=IndirectOffsetOnAxis(ap=rix[:, sl:sl + 1],
                                                           axis=0),
                            compute_op=mybir.AluOpType.add)
                    if debug_taps and sg == 0:
                        nc.sync.dma_start(xgdbg[:], xg[:, :, :])
                    # relu(x1) into a compact feature tile (labels dropped)
                    xf = xfp.tile([128, SLOTS, 64], dt.bfloat16, tag="xf")
                    nc.vector.tensor_scalar_max(xf[:, :, :], xg[:, :, 0:64],
                                                0.0)

                    # labels: [128, SLOTS] -> transpose -> dram row
                    lcol = lbp.tile([128, SLOTS], dt.bfloat16, tag="lcol")
                    nc.vector.tensor_copy(out=lcol[:], in_=xg[:, :, 64])
                    ltps = ltp.tile([128, 128], dt.bfloat16, tag="ltps")
                    nc.tensor.transpose(ltps[0:SLOTS, :], lcol[:], ident[:])
                    lso = lbp.tile([SLOTS, 128], dt.float32, tag="lso")
                    nc.scalar.copy(lso[:], ltps[0:SLOTS, :])
                    nc.sync.dma_start(
                        out=dram_view(labo, e0, [(128, SLOTS), (1, 128)]),
                        in_=lso[:])

                    for b in range(SLOTS // 16):  # 6 blocks of 2048 edges
                        tps = tpsp.tile([128, 1024], dt.bfloat16, tag="tps")
                        for k in range(8):
                            sl = b * 16 + 2 * k
                            pair = AP(xf[:].tensor,
                                      xf[:].offset + sl * 64,
                                      [(SLOTS * 64, 128), (1, 128)])
                            nc.tensor.transpose(
                                tps[:, k * 128:(k + 1) * 128], pair, ident[:])
                        x1t = x1tp.tile([128, 1024], dt.bfloat16, tag="x1t")
                        nc.any.tensor_copy(x1t[:], tps[:])

                        for j in range(2):
                            ch = b * 2 + j          # 1024-edge chunk in sg
                            y2 = y2p.tile([128, 512], dt.float32, tag="y2")
                            nc.tensor.matmul(y2[:], lhsT=w1t[:],
                                             rhs=x1t[:, j * 512:(j + 1) * 512],
                                             start=True, stop=True)
                            h2 = h2p.tile([128, 512], dt.bfloat16, tag="h2")
                            nc.scalar.activation(h2[:], y2[:], RELU,
                                                 bias=b1t[:])
                            pos = (ch % 4) * 32
                            if pos == 0:
                                sgps = sgp.tile([128, 512], dt.float32,
                                                tag="sgps")
                            nc.tensor.matmul(sgps[pos:pos + 32, :],
                                             lhsT=cbt[:], rhs=h2[:],
                                             start=True, stop=True,
                                             tile_position=(0, pos))
                            if pos == 96:
                                win = sg * WINDOWS_PER_SG + ch // 4
                                sout = sop.tile([128, 512], dt.float32,
                                                tag="sout")
                                nc.scalar.activation(sout[:], sgps[:], SIGM,
                                                     bias=sbt[:])
                                nc.sync.dma_start(out=oscr[win], in_=sout[:])

    nc.compile()
    return nc, e_pad


def _prep_inputs(pcd, feats, also_pts, also_labels, row, col,
                 w_in, b_in, w1, b1, w2, b2, w_out, b_out, n_sg=N_SG,
                 node_groups=NODE_GROUPS):
    """Host-side input prep: fold weights, cast/transpose tables, tile idx."""
    f32 = np.float32
    e_pad = n_sg * SG_EDGES
    npad = node_groups * 1024
    nn = feats.shape[0]
    mm = also_pts.shape[0]

    w_feat = w_in[:64].astype(f32)
    w_pos = w_in[64:67].astype(f32)

    featsT_ext = np.zeros((67, npad), BF16)
    featsT_ext[:64, :nn] = feats.T.astype(BF16)
    featsT_ext[64:67, :nn] = (-pcd.T).astype(BF16)
    wext = np.concatenate([w_feat, w_pos], 0).astype(BF16)

    alsoT = np.zeros((5, npad), BF16)
    alsoT[:3, :mm] = also_pts.T.astype(BF16)
    alsoT[3, :] = BF16(1.0)
    alsoT[4, :mm] = also_labels.astype(BF16)
    wq = np.zeros((5, ROW_Q), BF16)
    wq[0:3, 0:64] = w_pos.astype(BF16)
    wq[3, 0:64] = b_in.astype(BF16)
    wq[4, 64] = BF16(1.0)

    w1blk = np.zeros((128, 128), BF16)
    w1blk[0:64, 0:64] = w1.astype(BF16)
    w1blk[64:128, 64:128] = w1.astype(BF16)
    b1blk = np.concatenate([b1, b1]).astype(f32).reshape(128, 1)

    c = (w2.astype(f32) @ (w_out[:, 1] - w_out[:, 0]).astype(f32)).astype(f32)
    c0 = float(b2.astype(f32) @ (w_out[:, 1] - w_out[:, 0]).astype(f32)
               + b_out[1] - b_out[0])
    cblk = np.zeros((128, 32), BF16)
    cblk[0:64, 0] = (-c).astype(BF16)    # p0 of block A
    cblk[64:128, 1] = (-c).astype(BF16)  # p0 of block B
    cblk[0:64, 2] = c.astype(BF16)       # p1 of block A
    cblk[64:128, 3] = c.astype(BF16)     # p1 of block B
    sigbias = np.zeros((128, 1), f32)
    for p0 in (0, 32, 64, 96):
        sigbias[p0 + 0] = -c0
        sigbias[p0 + 1] = -c0
        sigbias[p0 + 2] = c0
        sigbias[p0 + 3] = c0

    def tile_idx(arr_shard):
        pad = np.zeros(e_pad, np.int32)
        pad[:len(arr_shard)] = arr_shard.astype(np.int32)
        return np.ascontiguousarray(
            pad.reshape(n_sg, SLOTS, 128).transpose(0, 2, 1))

    shared = dict(featsT=featsT_ext, alsoT=alsoT, wext=wext,
                  wq=wq, w1blk=w1blk, b1blk=b1blk, cblk=cblk, sigbias=sigbias)
    in_maps = []
    e_shard = E_SHARD if n_sg == N_SG else e_pad
    for cid in range(NCORES):
        lo = cid * e_shard
        hi = min(lo + e_shard, len(row))
        m = dict(shared)
        m["colx"] = tile_idx(col[lo:hi])
        m["rowx"] = tile_idx(row[lo:hi])
        in_maps.append(m)
    return in_maps, e_shard


def _extract_probs(oscr, e_pad):
    """oscr [n_win, 128, 512] -> probs [e_pad, 2] in edge order."""
    n_win = oscr.shape[0]
    # rows pos+0/1 = p0 of block A/B, pos+2/3 = p1; col = k*128+p
    o = oscr.reshape(n_win, 4, 32, 4, 128)[:, :, 0:4]  # [w, j2, r, k, p]
    p0 = o[:, :, 0:2]            # [w, j2, a, k, p]
    p1 = o[:, :, 2:4]
    # edge index = ((w*4 + j2)*4 + k)*256 + a*128 + p
    p0 = p0.transpose(0, 1, 3, 2, 4).reshape(-1)
    p1 = p1.transpose(0, 1, 3, 2, 4).reshape(-1)
    return np.stack([p0[:e_pad], p1[:e_pad]], 1)


def kernel(pcd, feats, also_pts, also_labels, row, col,
           w_in, b_in, w1, b1, w2, b2, w_out, b_out):
    from concourse.bass_utils import run_bass_kernel_spmd

    if "nc" not in _CACHE:
        _CACHE["nc"], _CACHE["e_pad"] = _build()
    nc = _CACHE["nc"]
    e_pad = _CACHE["e_pad"]

    in_maps, e_shard = _prep_inputs(
        pcd, feats, also_pts, also_labels, row, col,
        w_in, b_in, w1, b1, w2, b2, w_out, b_out)

    res = run_bass_kernel_spmd(nc, in_maps, core_ids=list(range(NCORES)))
    probs = np.empty((E, 2), np.float32)
    labels = np.empty((E,), np.int64)
    for cid in range(NCORES):
        pr = _extract_probs(res.results[cid]["oscr"], e_pad)
        lo = cid * E_SHARD
        probs[lo:lo + E_SHARD] = pr[:E_SHARD]
        labels[lo:lo + E_SHARD] = \
            res.results[cid]["labo"][:E_SHARD].astype(np.int64)
    return probs, labels
